# revision 64
# baseline (speedup 1.0000x reference)
"""Trainium2 Bass kernel for nn_CovarianceEstimator.

Computes, for y [B=16, R=1, A=16, T=14, S=1024] complex (given as separate
real/imag f32 tensors):
  - gather P=1024 pilot positions (sym_p, sc_p) from estimation_indices
  - per-position A x A outer products sig_p sig_p^H
  - unsorted-segment-mean over subcarrier ids sc_p
  - nearest-neighbor expand via closest_subcarrier to all S subcarriers
  - broadcast over T symbols
Output: [B, R, T, S, A, A] complex64.

Sharding: data-parallel over batch; 2 batches per core on 8 cores.

The reference's trailing broadcast_to over OFDM symbols is a zero-FLOP
replication (every t gets the same [S, A, A] covariance), so the device
computes and writes the covariance once per (batch, subcarrier) --
[B_LOC, 2(re/im), S, A*A] fp16 planes -- and the host assembles complex64
and returns a stride-0 numpy broadcast view over T.  This mirrors the
input side, where the host packs only the two pilot-symbol slabs instead
of shipping all 14 symbols to the device.

Device pipeline per batch b (all 4 even-subcarrier pairs per partition in
one merged chunk):
  psT[b][q, m, (ri a h)] = y[b, ., sym_h, 8q + 2m]      (PE transpose)
  sig tiles [q, h, m, a] = sqrt(1/2) * psT  (fp16)      (ACT, PSUM->SBUF)
  fre/fim[b][q, m, (i j)] = cov(s' = 4q + m)            (DVE)
  out[b, ri, 8q + 2m + e, :] = f*[b][q, m]              (DMA)
DVE math: products as fp16 1x ops; planar re/im output keeps every final
write dense so the h-sum adds hit the packed 2x/4x modes; the imaginary
part uses fim = R - R^T (R = K_h0 + K_h1, transposed-AP read); the
nearest-neighbor duplication e and the T broadcast are not materialised
on device (e rides a stride-0 DMA source dim, T a host broadcast view).

Two device-program builders:
  * fast path  - used when the index tensors match the PilotPattern structure
                 (meshgrid of 2 symbols x every-2nd-subcarrier, closest = even
                 floor).
  * generic    - any estimation_indices / closest_subcarrier.  Host folds the
                 whole segment-mean + NN-gather into one dense [S, P] weight
                 matrix applied on the tensor engine.
"""

import numpy as np

B, R, A, T, S = 16, 1, 16, 14, 1024
P_EST = 1024          # number of (sym, sc) estimation positions
N_CORES = 8
B_LOC = B // N_CORES  # 2 batches per core
AA2 = A * A * 2       # interleaved (re, im) row payload per subcarrier

_cache = {}


def _fast_path_info(est, closest):
    """Return (sym0, sym1) if indices match the pilot-pattern structure:
    est == meshgrid([sym0, sym1], arange(0, S, 2)) row-major and
    closest == 2*(arange(S)//2).  Else None."""
    if est.shape != (P_EST, 2) or closest.shape != (S,):
        return None
    sc = np.arange(0, S, 2, dtype=est.dtype)
    if not np.array_equal(est[: S // 2, 1], sc):
        return None
    if not np.array_equal(est[S // 2 :, 1], sc):
        return None
    sym0 = int(est[0, 0])
    sym1 = int(est[S // 2, 0])
    if not (0 <= sym0 < T and 0 <= sym1 < T):
        return None
    if sym1 <= sym0:
        return None  # keep the pilot symbols ordered; generic path covers the rest
    if not np.all(est[: S // 2, 0] == sym0):
        return None
    if not np.all(est[S // 2 :, 0] == sym1):
        return None
    if not np.array_equal(closest, (2 * (np.arange(S) // 2)).astype(closest.dtype)):
        return None
    return sym0, sym1


def _build_fast(sym0, sym1):
    import concourse.bacc as bacc
    import concourse.mybir as mybir
    from concourse.tile import TileContext

    f32 = mybir.dt.float32
    f16 = mybir.dt.float16
    nc = bacc.Bacc(trn_type="TRN2", target_bir_lowering=False)
    # y2t: host-packed, sqrt(1/2)-prescaled pilot slabs in transpose-ready
    # m-major blocks: [b, m, (h ri a), q] with subcarrier s' = 4q + m.
    # Row order (h, ri, a): slicing ri keeps (m h) memory-compact, so the
    # cross-product muls stay merged over (m h).
    y2t = nc.declare_dram_parameter("y2t", [B_LOC, 4, 64, 128], f16, isOutput=False)
    # out is (re, im)-planar: [b, ri, s, (i j)].  Planar keeps every DVE
    # write dense step-1 (the interleaved (ij ri) layout forces 1x mode);
    # the host assembles complex64 from the two planes.
    out = nc.declare_dram_parameter("out", [B_LOC, 2, S, A * A], f16, isOutput=True)

    M = S // 256    # 4 even-subcarrier pairs per partition

    with TileContext(nc) as tc:
        with (
            tc.tile_pool(name="const", bufs=1) as cp,
            tc.tile_pool(name="u", bufs=2) as up,
            tc.tile_pool(name="f", bufs=1) as fp,
        ):
            # DMA-transpose (HWDGE xbar) each m-block [64, 128] straight from
            # DRAM into the sig tile [q, m, ri, h, a]: subcarrier s' = 4q + m
            # lands in partition q with the (ri h a) row payload as columns.
            # No PE transpose, no PSUM, no ACT scale pass (sqrt(1/2) is
            # host-applied); batch-0 blocks split across both HWDGE queues.
            # Batch 0 lands in per-m-pair tiles with queue-alternated blocks,
            # so its first product mul starts after only two transposes; the
            # Tile dependency is then exact (tile granularity).
            sgT0 = [
                cp.tile([128, 2, 2, 2, A], f16, name=f"sgT0_{p}")
                for p in range(2)
            ]
            sgT1 = cp.tile([128, M, 2, 2, A], f16, name="sgT1")
            for m in range(M):
                (nc.sync, nc.scalar)[m % 2].dma_start(
                    out=sgT0[m // 2][:, m % 2],
                    in_=y2t[0, m],
                    transpose=True,
                )
            for m in range(M):
                (nc.sync, nc.scalar)[m % 2].dma_start(
                    out=sgT1[:, m],
                    in_=y2t[1, m],
                    transpose=True,
                )

            # DVE outer products (fp16).  Per batch, all 4 m's in one set of
            # merged ops [q, (h m), A, A]:
            #   re: u0 = sr (x) sr, u1 = si (x) si; w = u0 + u1 (2x);
            #       fre = w[h0] + w[h1]   (dense packed write)
            #   im: K = si (x) sr; R = K[h0] + K[h1] (2x);
            #       fim = R - R^T        (transposed-AP read, 1x)
            # The nearest-neighbor duplication is NOT materialised in SBUF:
            # the output DMA fans each row out to both subcarriers with a
            # stride-0 source dim, which keeps the final DVE writes dense
            # (packed mode) and halves their size.
            fre_t = [
                fp.tile([128, M, A * A], f16, name=f"fre{b}")
                for b in range(B_LOC)
            ]
            fim_t = [
                fp.tile([128, M, A * A], f16, name=f"fim{b}")
                for b in range(B_LOC)
            ]
            def va(x, rows, axis):  # sig tile view, (m ri h) flattened
                sh = [128, rows, A, A]
                v = x.rearrange("q m r h a -> q (m r h) a")
                return (
                    v[:, :, :, None].to_broadcast(sh)
                    if axis == 0
                    else v[:, :, None, :].to_broadcast(sh)
                )

            def vh(x, h, lo, hi):  # h-slice of w0 [q, m, h, (i j)]
                return x[:, lo:hi, h]

            for b in range(B_LOC):
                uu = up.tile([128, M, 2, 2, A, A], f16, tag="uu")
                kkt = up.tile([128, M, 2, A, A], f16, tag="kkt")
                w0 = up.tile([128, M, 2, A * A], f16, tag="w0")
                rr = up.tile([128, M, A, A], f16, tag="rr")
                if b == 0:
                    # per-pair muls: the first starts after only two
                    # transpose blocks have landed
                    for p in range(2):
                        nc.vector.tensor_mul(
                            uu[:, 2 * p : 2 * p + 2].rearrange(
                                "q m h r i j -> q (m h r) i j"
                            ),
                            va(sgT0[p][:], 8, 0),
                            va(sgT0[p][:], 8, 1),
                        )
                    for p in range(2):
                        nc.vector.tensor_mul(
                            kkt[:, 2 * p : 2 * p + 2].rearrange(
                                "q m h i j -> q (m h) i j"
                            ),
                            sgT0[p][:, :, :, 1, :]
                            .rearrange("q m h a -> q (m h) a")[:, :, :, None]
                            .to_broadcast([128, 4, A, A]),
                            sgT0[p][:, :, :, 0, :]
                            .rearrange("q m h a -> q (m h) a")[:, :, None, :]
                            .to_broadcast([128, 4, A, A]),
                        )
                else:
                    # one mul covers both sr (x) sr and si (x) si (ri-blocks)
                    nc.vector.tensor_mul(
                        uu[:].rearrange("q m h r i j -> q (m h r) i j"),
                        va(sgT1[:], 4 * M, 0),
                        va(sgT1[:], 4 * M, 1),
                    )
                    # one merged cross-product mul si (x) sr over (m h)
                    nc.vector.tensor_mul(
                        kkt[:].rearrange("q m h i j -> q (m h) i j"),
                        sgT1[:, :, :, 1, :]
                        .rearrange("q m h a -> q (m h) a")[:, :, :, None]
                        .to_broadcast([128, 2 * M, A, A]),
                        sgT1[:, :, :, 0, :]
                        .rearrange("q m h a -> q (m h) a")[:, :, None, :]
                        .to_broadcast([128, 2 * M, A, A]),
                    )
                # dense ri/h-sums hit the fp16 packed modes
                nc.vector.tensor_add(
                    w0[:],
                    uu[:, :, :, 0].rearrange("q m h i j -> q m h (i j)"),
                    uu[:, :, :, 1].rearrange("q m h i j -> q m h (i j)"),
                )
                nc.vector.tensor_add(rr[:], kkt[:, :, 0], kkt[:, :, 1])
                # Per-half final writes + per-plane DMAs so stores overlap the
                # next half's ops; out[b, ri, 8q + 2m + e, :] = f*[b][q, m]
                # (e-fanout via a stride-0 source dim in the DMA).
                # im rides the identity fim = R - R^T (transpose distributes
                # over the h-sum).
                H2 = M // 2
                dstr = out[b, 0].rearrange(
                    "(q n e) c -> q n (e c)", q=128, n=M, e=2
                )
                dsti = out[b, 1].rearrange(
                    "(q n e) c -> q n (e c)", q=128, n=M, e=2
                )
                qa = (nc.sync, nc.scalar)
                for g in range(2):
                    lo, hi = g * H2, (g + 1) * H2
                    # re-plane DMAs issue (and stream) while the im-plane
                    # subtract still runs, so only the im stores trail the
                    # final DVE op.
                    nc.vector.tensor_add(
                        fre_t[b][:, lo:hi], vh(w0, 0, lo, hi), vh(w0, 1, lo, hi)
                    )
                    for n in range(lo, hi):
                        qa[(b + n) % 2].dma_start(
                            out=dstr[:, n],
                            in_=fre_t[b][:, n, None, :].to_broadcast(
                                [128, 2, A * A]
                            ),
                        )
                    nc.vector.tensor_sub(
                        fim_t[b][:, lo:hi].rearrange(
                            "q n (i j) -> q n i j", i=A
                        ),
                        rr[:, lo:hi],
                        rr[:, lo:hi].rearrange("q n i j -> q n j i"),
                    )
                    for n in range(lo, hi):
                        qa[(b + n + 1) % 2].dma_start(
                            out=dsti[:, n],
                            in_=fim_t[b][:, n, None, :].to_broadcast(
                                [128, 2, A * A]
                            ),
                        )
    nc.finalize()
    return nc


def _build_generic(est, closest):
    """Generic program: host-gathered sig^T comes in as an input; the whole
    segment-mean + NN-gather is one dense weight matmul on the PE.
      cov[s, (i,j)] = sum_p wt[p, s] * G[p, (i,j)],  G from sig outer products.
    """
    import concourse.bacc as bacc
    import concourse.mybir as mybir
    from concourse.tile import TileContext

    f32 = mybir.dt.float32
    f16 = mybir.dt.float16
    nc = bacc.Bacc(trn_type="TRN2", target_bir_lowering=False)
    # sig^T per batch: [P_EST, A] split as [KP=8, 128, A]
    sgr = nc.declare_dram_parameter("sgr", [B_LOC, P_EST // 128, 128, A], f32, isOutput=False)
    sgi = nc.declare_dram_parameter("sgi", [B_LOC, P_EST // 128, 128, A], f32, isOutput=False)
    wt = nc.declare_dram_parameter("wt", [P_EST, S], f32, isOutput=False)
    out = nc.declare_dram_parameter("out", [B_LOC, 2, S, A * A], f16, isOutput=True)

    KP = P_EST // 128  # contraction chunks
    MS = S // 128      # output subcarrier chunks

    with TileContext(nc) as tc:
        with (
            tc.tile_pool(name="w", bufs=1) as wp,
            tc.tile_pool(name="sig", bufs=2) as sigp,
            tc.tile_pool(name="g", bufs=4) as gp,
            tc.tile_pool(name="ps", bufs=8, space="PSUM") as psp,
            tc.tile_pool(name="f", bufs=2) as fp,
        ):
            w_all = wp.tile([128, KP, S], f32, name="w_all")
            nc.sync.dma_start(
                out=w_all[:], in_=wt[:].rearrange("(k q) s -> q k s", k=KP, q=128)
            )
            for b in range(B_LOC):
                sr = sigp.tile([128, KP, A], f32, tag="sr")
                si = sigp.tile([128, KP, A], f32, tag="si")
                nc.sync.dma_start(
                    out=sr[:], in_=sgr[b].rearrange("k q a -> q k a")
                )
                nc.sync.dma_start(
                    out=si[:], in_=sgi[b].rearrange("k q a -> q k a")
                )

                f = fp.tile([128, 2, MS, A * A], f16, tag="f")
                gtiles = {}
                for k in range(KP):
                    def ii(x):
                        return x[:, k, :, None].to_broadcast([128, A, A])

                    def jj(x):
                        return x[:, k, None, :].to_broadcast([128, A, A])

                    gr = gp.tile([128, A, A], f32, tag=f"gr{k}")
                    gi = gp.tile([128, A, A], f32, tag=f"gi{k}")
                    tt = gp.tile([128, A, A], f32, tag="tt")
                    nc.vector.tensor_mul(gr[:], ii(sr), jj(sr))
                    nc.vector.tensor_mul(tt[:], ii(si), jj(si))
                    nc.vector.tensor_add(gr[:], gr[:], tt[:])
                    nc.vector.tensor_mul(gi[:], ii(si), jj(sr))
                    nc.vector.tensor_mul(tt[:], ii(sr), jj(si))
                    nc.vector.tensor_sub(gi[:], gi[:], tt[:])
                    gtiles[k] = (gr, gi)

                for m in range(MS):
                    for part in range(2):
                        pp = psp.tile([128, A * A], f32, tag="pp")
                        for k in range(KP):
                            g = gtiles[k][part]
                            nc.tensor.matmul(
                                pp[:],
                                lhsT=w_all[:, k, m * 128 : (m + 1) * 128],
                                rhs=g[:].rearrange("q i j -> q (i j)"),
                                start=(k == 0),
                                stop=(k == KP - 1),
                            )
                        nc.vector.tensor_copy(f[:, part, m, :], pp[:])

                dst = out[b].rearrange(
                    "ri (m q) ij -> q ri m ij", m=MS, q=128
                )
                nc.sync.dma_start(out=dst, in_=f[:])
    nc.finalize()
    return nc


def _get_program(est, closest):
    key = (est.tobytes(), closest.tobytes())
    hit = _cache.get(key)
    if hit is not None:
        return hit
    fast = _fast_path_info(est, closest)
    if fast is not None:
        prog = ("fast", _build_fast(*fast), None)
    else:
        counts = np.zeros(S, dtype=np.float64)
        np.add.at(counts, est[:, 1], 1.0)
        denom = np.maximum(counts, 1.0)
        # wt[p, s] = [sc_p == closest[s]] / denom[closest[s]]
        wt = (
            (est[:, 1][:, None] == closest[None, :]).astype(np.float32)
            / denom[closest][None, :].astype(np.float32)
        )
        prog = ("generic", _build_generic(est, closest), np.ascontiguousarray(wt))
    _cache[key] = prog
    return prog


def _make_in_maps(inputs, est, kind, wt):
    yr = np.ascontiguousarray(np.asarray(inputs["y_real"], dtype=np.float32)[:, 0])
    yi = np.ascontiguousarray(np.asarray(inputs["y_imag"], dtype=np.float32)[:, 0])
    if kind == "fast":
        sym0, sym1 = int(est[0, 0]), int(est[P_EST // 2, 0])
        # pack [B, m, (ri h a), q] fp16: the two pilot-symbol slabs at the
        # even (estimated) subcarriers, sqrt(1/2)-prescaled, reordered into
        # per-m DMA-transpose blocks (s' = 4q + m)
        sl = np.stack(
            [yr[:, :, (sym0, sym1), 0::2], yi[:, :, (sym0, sym1), 0::2]],
            axis=1,
        ) * np.float32(0.7071067811865476)  # [B, ri, A, h, 512]
        y2t = np.ascontiguousarray(
            sl.reshape(B, 2, A, 2, 128, 4)
            .transpose(0, 5, 3, 1, 2, 4)  # [B, m, h, ri, A, q]
            .reshape(B, 4, 64, 128)
            .astype(np.float16)
        )
        return [
            {"y2t": y2t[c * B_LOC : (c + 1) * B_LOC]} for c in range(N_CORES)
        ]
    sym = est[:, 0].astype(np.int64)
    sc = est[:, 1].astype(np.int64)
    # host gather: sig[b, a, p] = y[b, a, sym_p, sc_p]
    sgr = yr[:, :, sym, sc]  # [B, A, P]
    sgi = yi[:, :, sym, sc]
    # -> [B, KP, 128, A]
    sgr = np.ascontiguousarray(
        sgr.transpose(0, 2, 1).reshape(B, P_EST // 128, 128, A)
    )
    sgi = np.ascontiguousarray(
        sgi.transpose(0, 2, 1).reshape(B, P_EST // 128, 128, A)
    )
    return [
        {
            "sgr": sgr[c * B_LOC : (c + 1) * B_LOC],
            "sgi": sgi[c * B_LOC : (c + 1) * B_LOC],
            "wt": wt,
        }
        for c in range(N_CORES)
    ]


def kernel(y_real, y_imag, estimation_indices, closest_subcarrier):
    from concourse.bass_utils import run_bass_kernel_spmd

    assert y_real.shape == (B, R, A, T, S), y_real.shape
    est = np.asarray(estimation_indices)
    closest = np.asarray(closest_subcarrier)
    kind, nc, wt = _get_program(est, closest)
    in_maps = _make_in_maps(
        {"y_real": y_real, "y_imag": y_imag}, est, kind, wt
    )

    res = run_bass_kernel_spmd(nc, in_maps, list(range(N_CORES)))
    parts = [np.asarray(res.results[c]["out"]) for c in range(N_CORES)]
    full = np.concatenate(parts, axis=0)  # [B, 2, S, A*A] fp16 (re, im planes)
    cov = np.empty((B, S, A * A), dtype=np.complex64)
    cov.real = full[:, 0]
    cov.imag = full[:, 1]
    cov = cov.reshape(B, R, 1, S, A, A)
    # The per-symbol covariance is t-independent: broadcast over T as a view.
    return np.broadcast_to(cov, (B, R, T, S, A, A))


# revision 65
# speedup vs baseline: 1.0032x; 1.0032x over previous
"""Trainium2 Bass kernel for nn_CovarianceEstimator.

Computes, for y [B=16, R=1, A=16, T=14, S=1024] complex (given as separate
real/imag f32 tensors):
  - gather P=1024 pilot positions (sym_p, sc_p) from estimation_indices
  - per-position A x A outer products sig_p sig_p^H
  - unsorted-segment-mean over subcarrier ids sc_p
  - nearest-neighbor expand via closest_subcarrier to all S subcarriers
  - broadcast over T symbols
Output: [B, R, T, S, A, A] complex64.

Sharding: data-parallel over batch; 2 batches per core on 8 cores.

The reference's trailing broadcast_to over OFDM symbols is a zero-FLOP
replication (every t gets the same [S, A, A] covariance), so the device
computes and writes the covariance once per (batch, subcarrier) --
[B_LOC, 2(re/im), S, A*A] fp16 planes -- and the host assembles complex64
and returns a stride-0 numpy broadcast view over T.  This mirrors the
input side, where the host packs only the two pilot-symbol slabs instead
of shipping all 14 symbols to the device.

Device pipeline per batch b:
  sgT[b][q, m, (h ri a)] = sqrt(1/2) y[b, ., sym_h, 8q+2m]  (DMA-transpose,
      host-packed m-major [64, 128] fp16 blocks through the HWDGE xbar --
      no PE transpose, PSUM or ACT pass; batch 0 lands in per-m-pair tiles
      with queue-alternated blocks so its first mul starts after only two)
  fre/fim[b][q, m, (i j)] = cov(s' = 4q + m)                (DVE)
  out[b, ri, 8q + 2m + e, :] = f*[b][q, m]                  (DMA)
DVE math: outer products as fp16 1x ops (one merged (m h ri)-row mul for
the re-part pair, (m h)-merged muls for the si (x) sr cross products --
the (h, ri, a) payload row order keeps (m h) compact after slicing ri);
planar re/im output keeps every final write dense so the ri/h-sum adds hit
the packed 2x/4x modes; the imaginary part uses fim = R - R^T (R = K_h0 +
K_h1, transposed-AP read); the nearest-neighbor duplication e and the T
broadcast are not materialised on device (e rides a stride-0 DMA source
dim, T a host broadcast view).

Two device-program builders:
  * fast path  - used when the index tensors match the PilotPattern structure
                 (meshgrid of 2 symbols x every-2nd-subcarrier, closest = even
                 floor).
  * generic    - any estimation_indices / closest_subcarrier.  Host folds the
                 whole segment-mean + NN-gather into one dense [S, P] weight
                 matrix applied on the tensor engine.
"""

import numpy as np

B, R, A, T, S = 16, 1, 16, 14, 1024
P_EST = 1024          # number of (sym, sc) estimation positions
N_CORES = 8
B_LOC = B // N_CORES  # 2 batches per core
AA2 = A * A * 2       # interleaved (re, im) row payload per subcarrier

_cache = {}


def _fast_path_info(est, closest):
    """Return (sym0, sym1) if indices match the pilot-pattern structure:
    est == meshgrid([sym0, sym1], arange(0, S, 2)) row-major and
    closest == 2*(arange(S)//2).  Else None."""
    if est.shape != (P_EST, 2) or closest.shape != (S,):
        return None
    sc = np.arange(0, S, 2, dtype=est.dtype)
    if not np.array_equal(est[: S // 2, 1], sc):
        return None
    if not np.array_equal(est[S // 2 :, 1], sc):
        return None
    sym0 = int(est[0, 0])
    sym1 = int(est[S // 2, 0])
    if not (0 <= sym0 < T and 0 <= sym1 < T):
        return None
    if sym1 <= sym0:
        return None  # keep the pilot symbols ordered; generic path covers the rest
    if not np.all(est[: S // 2, 0] == sym0):
        return None
    if not np.all(est[S // 2 :, 0] == sym1):
        return None
    if not np.array_equal(closest, (2 * (np.arange(S) // 2)).astype(closest.dtype)):
        return None
    return sym0, sym1


def _build_fast(sym0, sym1):
    import concourse.bacc as bacc
    import concourse.mybir as mybir
    from concourse.tile import TileContext

    f32 = mybir.dt.float32
    f16 = mybir.dt.float16
    nc = bacc.Bacc(trn_type="TRN2", target_bir_lowering=False)
    # y2t: host-packed, sqrt(1/2)-prescaled pilot slabs in transpose-ready
    # m-major blocks: [b, m, (h ri a), q] with subcarrier s' = 4q + m.
    # Row order (h, ri, a): slicing ri keeps (m h) memory-compact, so the
    # cross-product muls stay merged over (m h).
    y2t = nc.declare_dram_parameter("y2t", [B_LOC, 4, 64, 128], f16, isOutput=False)
    # out is (re, im)-planar: [b, ri, s, (i j)].  Planar keeps every DVE
    # write dense step-1 (the interleaved (ij ri) layout forces 1x mode);
    # the host assembles complex64 from the two planes.
    out = nc.declare_dram_parameter("out", [B_LOC, 2, S, A * A], f16, isOutput=True)

    M = S // 256    # 4 even-subcarrier pairs per partition

    with TileContext(nc) as tc:
        with (
            tc.tile_pool(name="const", bufs=1) as cp,
            tc.tile_pool(name="u", bufs=2) as up,
            tc.tile_pool(name="f", bufs=1) as fp,
        ):
            # DMA-transpose (HWDGE xbar) each m-block [64, 128] straight from
            # DRAM into the sig tile [q, m, ri, h, a]: subcarrier s' = 4q + m
            # lands in partition q with the (ri h a) row payload as columns.
            # No PE transpose, no PSUM, no ACT scale pass (sqrt(1/2) is
            # host-applied); batch-0 blocks split across both HWDGE queues.
            # Batch 0 lands in per-m-pair tiles with queue-alternated blocks,
            # so its first product mul starts after only two transposes; the
            # Tile dependency is then exact (tile granularity).
            sgT0 = [
                cp.tile([128, 2, 2, 2, A], f16, name=f"sgT0_{p}")
                for p in range(2)
            ]
            sgT1 = cp.tile([128, M, 2, 2, A], f16, name="sgT1")
            for m in range(M):
                (nc.sync, nc.scalar)[m % 2].dma_start(
                    out=sgT0[m // 2][:, m % 2],
                    in_=y2t[0, m],
                    transpose=True,
                )
            for m in range(M):
                (nc.sync, nc.scalar)[m % 2].dma_start(
                    out=sgT1[:, m],
                    in_=y2t[1, m],
                    transpose=True,
                )

            # DVE outer products (fp16).  Per batch, all 4 m's in one set of
            # merged ops [q, (h m), A, A]:
            #   re: u0 = sr (x) sr, u1 = si (x) si; w = u0 + u1 (2x);
            #       fre = w[h0] + w[h1]   (dense packed write)
            #   im: K = si (x) sr; R = K[h0] + K[h1] (2x);
            #       fim = R - R^T        (transposed-AP read, 1x)
            # The nearest-neighbor duplication is NOT materialised in SBUF:
            # the output DMA fans each row out to both subcarriers with a
            # stride-0 source dim, which keeps the final DVE writes dense
            # (packed mode) and halves their size.
            fre_t = [
                fp.tile([128, M, A * A], f16, name=f"fre{b}")
                for b in range(B_LOC)
            ]
            fim_t = [
                fp.tile([128, M, A * A], f16, name=f"fim{b}")
                for b in range(B_LOC)
            ]
            def va(x, rows, axis):  # sig tile view, (m ri h) flattened
                sh = [128, rows, A, A]
                v = x.rearrange("q m r h a -> q (m r h) a")
                return (
                    v[:, :, :, None].to_broadcast(sh)
                    if axis == 0
                    else v[:, :, None, :].to_broadcast(sh)
                )

            def vh(x, h, lo, hi):  # h-slice of w0 [q, m, h, (i j)]
                return x[:, lo:hi, h]

            for b in range(B_LOC):
                uu = up.tile([128, M, 2, 2, A, A], f16, tag="uu")
                kkt = up.tile([128, M, 2, A, A], f16, tag="kkt")
                w0 = up.tile([128, M, 2, A * A], f16, tag="w0")
                rr = up.tile([128, M, A, A], f16, tag="rr")
                if b == 0:
                    # per-pair muls: the first starts after only two
                    # transpose blocks have landed
                    for p in range(2):
                        nc.vector.tensor_mul(
                            uu[:, 2 * p : 2 * p + 2].rearrange(
                                "q m h r i j -> q (m h r) i j"
                            ),
                            va(sgT0[p][:], 8, 0),
                            va(sgT0[p][:], 8, 1),
                        )
                    for p in range(2):
                        nc.vector.tensor_mul(
                            kkt[:, 2 * p : 2 * p + 2].rearrange(
                                "q m h i j -> q (m h) i j"
                            ),
                            sgT0[p][:, :, :, 1, :]
                            .rearrange("q m h a -> q (m h) a")[:, :, :, None]
                            .to_broadcast([128, 4, A, A]),
                            sgT0[p][:, :, :, 0, :]
                            .rearrange("q m h a -> q (m h) a")[:, :, None, :]
                            .to_broadcast([128, 4, A, A]),
                        )
                else:
                    # one mul covers both sr (x) sr and si (x) si (ri-blocks)
                    nc.vector.tensor_mul(
                        uu[:].rearrange("q m h r i j -> q (m h r) i j"),
                        va(sgT1[:], 4 * M, 0),
                        va(sgT1[:], 4 * M, 1),
                    )
                    # one merged cross-product mul si (x) sr over (m h)
                    nc.vector.tensor_mul(
                        kkt[:].rearrange("q m h i j -> q (m h) i j"),
                        sgT1[:, :, :, 1, :]
                        .rearrange("q m h a -> q (m h) a")[:, :, :, None]
                        .to_broadcast([128, 2 * M, A, A]),
                        sgT1[:, :, :, 0, :]
                        .rearrange("q m h a -> q (m h) a")[:, :, None, :]
                        .to_broadcast([128, 2 * M, A, A]),
                    )
                # dense ri/h-sums hit the fp16 packed modes
                nc.vector.tensor_add(
                    w0[:],
                    uu[:, :, :, 0].rearrange("q m h i j -> q m h (i j)"),
                    uu[:, :, :, 1].rearrange("q m h i j -> q m h (i j)"),
                )
                nc.vector.tensor_add(rr[:], kkt[:, :, 0], kkt[:, :, 1])
                # Per-half final writes + per-plane DMAs so stores overlap the
                # next half's ops; out[b, ri, 8q + 2m + e, :] = f*[b][q, m]
                # (e-fanout via a stride-0 source dim in the DMA).
                # im rides the identity fim = R - R^T (transpose distributes
                # over the h-sum).
                H2 = M // 2
                dstr = out[b, 0].rearrange(
                    "(q n e) c -> q n (e c)", q=128, n=M, e=2
                )
                dsti = out[b, 1].rearrange(
                    "(q n e) c -> q n (e c)", q=128, n=M, e=2
                )
                qa = (nc.sync, nc.scalar)
                for g in range(2):
                    lo, hi = g * H2, (g + 1) * H2
                    # re-plane DMAs issue (and stream) while the im-plane
                    # subtract still runs, so only the im stores trail the
                    # final DVE op.
                    nc.vector.tensor_add(
                        fre_t[b][:, lo:hi], vh(w0, 0, lo, hi), vh(w0, 1, lo, hi)
                    )
                    for n in range(lo, hi):
                        qa[(b + n) % 2].dma_start(
                            out=dstr[:, n],
                            in_=fre_t[b][:, n, None, :].to_broadcast(
                                [128, 2, A * A]
                            ),
                        )
                    nc.vector.tensor_sub(
                        fim_t[b][:, lo:hi].rearrange(
                            "q n (i j) -> q n i j", i=A
                        ),
                        rr[:, lo:hi],
                        rr[:, lo:hi].rearrange("q n i j -> q n j i"),
                    )
                    for n in range(lo, hi):
                        qa[(b + n + 1) % 2].dma_start(
                            out=dsti[:, n],
                            in_=fim_t[b][:, n, None, :].to_broadcast(
                                [128, 2, A * A]
                            ),
                        )
    nc.finalize()
    return nc


def _build_generic(est, closest):
    """Generic program: host-gathered sig^T comes in as an input; the whole
    segment-mean + NN-gather is one dense weight matmul on the PE.
      cov[s, (i,j)] = sum_p wt[p, s] * G[p, (i,j)],  G from sig outer products.
    """
    import concourse.bacc as bacc
    import concourse.mybir as mybir
    from concourse.tile import TileContext

    f32 = mybir.dt.float32
    f16 = mybir.dt.float16
    nc = bacc.Bacc(trn_type="TRN2", target_bir_lowering=False)
    # sig^T per batch: [P_EST, A] split as [KP=8, 128, A]
    sgr = nc.declare_dram_parameter("sgr", [B_LOC, P_EST // 128, 128, A], f32, isOutput=False)
    sgi = nc.declare_dram_parameter("sgi", [B_LOC, P_EST // 128, 128, A], f32, isOutput=False)
    wt = nc.declare_dram_parameter("wt", [P_EST, S], f32, isOutput=False)
    out = nc.declare_dram_parameter("out", [B_LOC, 2, S, A * A], f16, isOutput=True)

    KP = P_EST // 128  # contraction chunks
    MS = S // 128      # output subcarrier chunks

    with TileContext(nc) as tc:
        with (
            tc.tile_pool(name="w", bufs=1) as wp,
            tc.tile_pool(name="sig", bufs=2) as sigp,
            tc.tile_pool(name="g", bufs=4) as gp,
            tc.tile_pool(name="ps", bufs=8, space="PSUM") as psp,
            tc.tile_pool(name="f", bufs=2) as fp,
        ):
            w_all = wp.tile([128, KP, S], f32, name="w_all")
            nc.sync.dma_start(
                out=w_all[:], in_=wt[:].rearrange("(k q) s -> q k s", k=KP, q=128)
            )
            for b in range(B_LOC):
                sr = sigp.tile([128, KP, A], f32, tag="sr")
                si = sigp.tile([128, KP, A], f32, tag="si")
                nc.sync.dma_start(
                    out=sr[:], in_=sgr[b].rearrange("k q a -> q k a")
                )
                nc.sync.dma_start(
                    out=si[:], in_=sgi[b].rearrange("k q a -> q k a")
                )

                f = fp.tile([128, 2, MS, A * A], f16, tag="f")
                gtiles = {}
                for k in range(KP):
                    def ii(x):
                        return x[:, k, :, None].to_broadcast([128, A, A])

                    def jj(x):
                        return x[:, k, None, :].to_broadcast([128, A, A])

                    gr = gp.tile([128, A, A], f32, tag=f"gr{k}")
                    gi = gp.tile([128, A, A], f32, tag=f"gi{k}")
                    tt = gp.tile([128, A, A], f32, tag="tt")
                    nc.vector.tensor_mul(gr[:], ii(sr), jj(sr))
                    nc.vector.tensor_mul(tt[:], ii(si), jj(si))
                    nc.vector.tensor_add(gr[:], gr[:], tt[:])
                    nc.vector.tensor_mul(gi[:], ii(si), jj(sr))
                    nc.vector.tensor_mul(tt[:], ii(sr), jj(si))
                    nc.vector.tensor_sub(gi[:], gi[:], tt[:])
                    gtiles[k] = (gr, gi)

                for m in range(MS):
                    for part in range(2):
                        pp = psp.tile([128, A * A], f32, tag="pp")
                        for k in range(KP):
                            g = gtiles[k][part]
                            nc.tensor.matmul(
                                pp[:],
                                lhsT=w_all[:, k, m * 128 : (m + 1) * 128],
                                rhs=g[:].rearrange("q i j -> q (i j)"),
                                start=(k == 0),
                                stop=(k == KP - 1),
                            )
                        nc.vector.tensor_copy(f[:, part, m, :], pp[:])

                dst = out[b].rearrange(
                    "ri (m q) ij -> q ri m ij", m=MS, q=128
                )
                nc.sync.dma_start(out=dst, in_=f[:])
    nc.finalize()
    return nc


def _get_program(est, closest):
    key = (est.tobytes(), closest.tobytes())
    hit = _cache.get(key)
    if hit is not None:
        return hit
    fast = _fast_path_info(est, closest)
    if fast is not None:
        prog = ("fast", _build_fast(*fast), None)
    else:
        counts = np.zeros(S, dtype=np.float64)
        np.add.at(counts, est[:, 1], 1.0)
        denom = np.maximum(counts, 1.0)
        # wt[p, s] = [sc_p == closest[s]] / denom[closest[s]]
        wt = (
            (est[:, 1][:, None] == closest[None, :]).astype(np.float32)
            / denom[closest][None, :].astype(np.float32)
        )
        prog = ("generic", _build_generic(est, closest), np.ascontiguousarray(wt))
    _cache[key] = prog
    return prog


def _make_in_maps(inputs, est, kind, wt):
    yr = np.ascontiguousarray(np.asarray(inputs["y_real"], dtype=np.float32)[:, 0])
    yi = np.ascontiguousarray(np.asarray(inputs["y_imag"], dtype=np.float32)[:, 0])
    if kind == "fast":
        sym0, sym1 = int(est[0, 0]), int(est[P_EST // 2, 0])
        # pack [B, m, (ri h a), q] fp16: the two pilot-symbol slabs at the
        # even (estimated) subcarriers, sqrt(1/2)-prescaled, reordered into
        # per-m DMA-transpose blocks (s' = 4q + m)
        sl = np.stack(
            [yr[:, :, (sym0, sym1), 0::2], yi[:, :, (sym0, sym1), 0::2]],
            axis=1,
        ) * np.float32(0.7071067811865476)  # [B, ri, A, h, 512]
        y2t = np.ascontiguousarray(
            sl.reshape(B, 2, A, 2, 128, 4)
            .transpose(0, 5, 3, 1, 2, 4)  # [B, m, h, ri, A, q]
            .reshape(B, 4, 64, 128)
            .astype(np.float16)
        )
        return [
            {"y2t": y2t[c * B_LOC : (c + 1) * B_LOC]} for c in range(N_CORES)
        ]
    sym = est[:, 0].astype(np.int64)
    sc = est[:, 1].astype(np.int64)
    # host gather: sig[b, a, p] = y[b, a, sym_p, sc_p]
    sgr = yr[:, :, sym, sc]  # [B, A, P]
    sgi = yi[:, :, sym, sc]
    # -> [B, KP, 128, A]
    sgr = np.ascontiguousarray(
        sgr.transpose(0, 2, 1).reshape(B, P_EST // 128, 128, A)
    )
    sgi = np.ascontiguousarray(
        sgi.transpose(0, 2, 1).reshape(B, P_EST // 128, 128, A)
    )
    return [
        {
            "sgr": sgr[c * B_LOC : (c + 1) * B_LOC],
            "sgi": sgi[c * B_LOC : (c + 1) * B_LOC],
            "wt": wt,
        }
        for c in range(N_CORES)
    ]


def kernel(y_real, y_imag, estimation_indices, closest_subcarrier):
    from concourse.bass_utils import run_bass_kernel_spmd

    assert y_real.shape == (B, R, A, T, S), y_real.shape
    est = np.asarray(estimation_indices)
    closest = np.asarray(closest_subcarrier)
    kind, nc, wt = _get_program(est, closest)
    in_maps = _make_in_maps(
        {"y_real": y_real, "y_imag": y_imag}, est, kind, wt
    )

    res = run_bass_kernel_spmd(nc, in_maps, list(range(N_CORES)))
    parts = [np.asarray(res.results[c]["out"]) for c in range(N_CORES)]
    full = np.concatenate(parts, axis=0)  # [B, 2, S, A*A] fp16 (re, im planes)
    cov = np.empty((B, S, A * A), dtype=np.complex64)
    cov.real = full[:, 0]
    cov.imag = full[:, 1]
    cov = cov.reshape(B, R, 1, S, A, A)
    # The per-symbol covariance is t-independent: broadcast over T as a view.
    return np.broadcast_to(cov, (B, R, T, S, A, A))


# revision 67
# speedup vs baseline: 1.0298x; 1.0265x over previous
"""Trainium2 Bass kernel for nn_CovarianceEstimator.

Computes, for y [B=16, R=1, A=16, T=14, S=1024] complex (given as separate
real/imag f32 tensors):
  - gather P=1024 pilot positions (sym_p, sc_p) from estimation_indices
  - per-position A x A outer products sig_p sig_p^H
  - unsorted-segment-mean over subcarrier ids sc_p
  - nearest-neighbor expand via closest_subcarrier to all S subcarriers
  - broadcast over T symbols
Output: [B, R, T, S, A, A] complex64.

Sharding: data-parallel over batch; 2 batches per core on 8 cores.

The reference's trailing broadcast_to over OFDM symbols is a zero-FLOP
replication (every t gets the same [S, A, A] covariance), so the device
computes and writes the covariance once per (batch, subcarrier) --
[B_LOC, 2(re/im), S, A*A] fp16 planes -- and the host assembles complex64
and returns a stride-0 numpy broadcast view over T.  This mirrors the
input side, where the host packs only the two pilot-symbol slabs instead
of shipping all 14 symbols to the device.

Device pipeline per batch b:
  sgT[b][q, m, (h ri a)] = sqrt(1/2) y[b, ., sym_h, 8q+2m]  (DMA-transpose,
      host-packed m-major [64, 128] fp16 blocks through the HWDGE xbar --
      no PE transpose, PSUM or ACT pass; batch 0 lands in per-m-pair tiles
      with queue-alternated blocks so its first mul starts after only two)
  fre/fim[b][q, m, (i j)] = cov(s' = 4q + m)                (DVE)
  out[b, ri, 8q + 2m + e, :] = f*[b][q, m]                  (DMA)
DVE math: outer products as fp16 1x ops (one merged (m h ri)-row mul for
the re-part pair, (m h)-merged muls for the si (x) sr cross products --
the (h, ri, a) payload row order keeps (m h) compact after slicing ri);
planar re/im output keeps every final write dense so the ri/h-sum adds hit
the packed 2x/4x modes; the imaginary part uses fim = R - R^T (R = K_h0 +
K_h1, transposed-AP read); the nearest-neighbor duplication e and the T
broadcast are not materialised on device (e rides a stride-0 DMA source
dim, T a host broadcast view).

Two device-program builders:
  * fast path  - used when the index tensors match the PilotPattern structure
                 (meshgrid of 2 symbols x every-2nd-subcarrier, closest = even
                 floor).
  * generic    - any estimation_indices / closest_subcarrier.  Host folds the
                 whole segment-mean + NN-gather into one dense [S, P] weight
                 matrix applied on the tensor engine.
"""

import numpy as np

B, R, A, T, S = 16, 1, 16, 14, 1024
P_EST = 1024          # number of (sym, sc) estimation positions
N_CORES = 8
B_LOC = B // N_CORES  # 2 batches per core
AA2 = A * A * 2       # interleaved (re, im) row payload per subcarrier

_cache = {}


def _fast_path_info(est, closest):
    """Return (sym0, sym1) if indices match the pilot-pattern structure:
    est == meshgrid([sym0, sym1], arange(0, S, 2)) row-major and
    closest == 2*(arange(S)//2).  Else None."""
    if est.shape != (P_EST, 2) or closest.shape != (S,):
        return None
    sc = np.arange(0, S, 2, dtype=est.dtype)
    if not np.array_equal(est[: S // 2, 1], sc):
        return None
    if not np.array_equal(est[S // 2 :, 1], sc):
        return None
    sym0 = int(est[0, 0])
    sym1 = int(est[S // 2, 0])
    if not (0 <= sym0 < T and 0 <= sym1 < T):
        return None
    if sym1 <= sym0:
        return None  # keep the pilot symbols ordered; generic path covers the rest
    if not np.all(est[: S // 2, 0] == sym0):
        return None
    if not np.all(est[S // 2 :, 0] == sym1):
        return None
    if not np.array_equal(closest, (2 * (np.arange(S) // 2)).astype(closest.dtype)):
        return None
    return sym0, sym1


def _build_fast(sym0, sym1):
    import concourse.bacc as bacc
    import concourse.mybir as mybir
    from concourse.tile import TileContext

    f32 = mybir.dt.float32
    f16 = mybir.dt.float16
    nc = bacc.Bacc(trn_type="TRN2", target_bir_lowering=False)
    # y2t: host-packed, sqrt(1/2)-prescaled pilot slabs in transpose-ready
    # m-major blocks: [b, m, (h ri a), q] with subcarrier s' = 4q + m.
    # Row order (h, ri, a): slicing ri keeps (m h) memory-compact, so the
    # cross-product muls stay merged over (m h).
    y2t = nc.declare_dram_parameter("y2t", [B_LOC, 4, 64, 128], f16, isOutput=False)
    # out is (re, im)-planar: [b, ri, s, (i j)].  Planar keeps every DVE
    # write dense step-1 (the interleaved (ij ri) layout forces 1x mode);
    # the host assembles complex64 from the two planes.
    out = nc.declare_dram_parameter("out", [B_LOC, 2, S, A * A], f16, isOutput=True)

    M = S // 256    # 4 even-subcarrier pairs per partition

    with TileContext(nc) as tc:
        with (
            tc.tile_pool(name="const", bufs=1) as cp,
            tc.tile_pool(name="u", bufs=2) as up,
            tc.tile_pool(name="f", bufs=1) as fp,
        ):
            # DMA-transpose (HWDGE xbar) each m-block [64, 128] straight from
            # DRAM into the sig tile [q, m, ri, h, a]: subcarrier s' = 4q + m
            # lands in partition q with the (ri h a) row payload as columns.
            # No PE transpose, no PSUM, no ACT scale pass (sqrt(1/2) is
            # host-applied); batch-0 blocks split across both HWDGE queues.
            # Batch 0 lands in per-m-pair tiles with queue-alternated blocks,
            # so its first product mul starts after only two transposes; the
            # Tile dependency is then exact (tile granularity).
            sgT0 = [
                cp.tile([128, 2, 2, 2, A], f16, name=f"sgT0_{p}")
                for p in range(2)
            ]
            sgT1 = cp.tile([128, M, 2, 2, A], f16, name="sgT1")
            for m in range(M):
                (nc.sync, nc.scalar)[m % 2].dma_start(
                    out=sgT0[m // 2][:, m % 2],
                    in_=y2t[0, m],
                    transpose=True,
                )
            for m in range(M):
                (nc.sync, nc.scalar)[m % 2].dma_start(
                    out=sgT1[:, m],
                    in_=y2t[1, m],
                    transpose=True,
                )

            # DVE outer products (fp16).  Per batch, all 4 m's in one set of
            # merged ops [q, (h m), A, A]:
            #   re: u0 = sr (x) sr, u1 = si (x) si; w = u0 + u1 (2x);
            #       fre = w[h0] + w[h1]   (dense packed write)
            #   im: K = si (x) sr; R = K[h0] + K[h1] (2x);
            #       fim = R - R^T        (transposed-AP read, 1x)
            # The nearest-neighbor duplication is NOT materialised in SBUF:
            # the output DMA fans each row out to both subcarriers with a
            # stride-0 source dim, which keeps the final DVE writes dense
            # (packed mode) and halves their size.
            fre_t = [
                fp.tile([128, M, A * A], f16, name=f"fre{b}")
                for b in range(B_LOC)
            ]
            fim_t = [
                fp.tile([128, M, A * A], f16, name=f"fim{b}")
                for b in range(B_LOC)
            ]
            def va(x, rows, axis):  # sig tile view, (m ri h) flattened
                sh = [128, rows, A, A]
                v = x.rearrange("q m r h a -> q (m r h) a")
                return (
                    v[:, :, :, None].to_broadcast(sh)
                    if axis == 0
                    else v[:, :, None, :].to_broadcast(sh)
                )

            def vh(x, h, lo, hi):  # h-slice of w0 [q, m, h, (i j)]
                return x[:, lo:hi, h]

            for b in range(B_LOC):
                uu = up.tile([128, M, 2, 2, A, A], f16, tag="uu")
                kkt = up.tile([128, M, 2, A, A], f16, tag="kkt")
                w0 = up.tile([128, M, 2, A * A], f16, tag="w0")
                rr = up.tile([128, M, A, A], f16, tag="rr")
                rrT = up.tile([128, M, A, A], f16, tag="rrT")
                # Emission order computes rr as early as possible so the
                # otherwise-idle ACT engine materialises R^T (strided
                # transposed copy) in parallel with the remaining DVE muls;
                # the fim subtracts then run dense in packed mode instead of
                # paying the 1.43x transposed-read penalty on DVE.
                if b == 0:
                    # per-pair muls: the first starts after only two
                    # transpose blocks have landed
                    nc.vector.tensor_mul(
                        uu[:, 0:2].rearrange("q m h r i j -> q (m h r) i j"),
                        va(sgT0[0][:], 8, 0),
                        va(sgT0[0][:], 8, 1),
                    )
                    for p in range(2):
                        nc.vector.tensor_mul(
                            kkt[:, 2 * p : 2 * p + 2].rearrange(
                                "q m h i j -> q (m h) i j"
                            ),
                            sgT0[p][:, :, :, 1, :]
                            .rearrange("q m h a -> q (m h) a")[:, :, :, None]
                            .to_broadcast([128, 4, A, A]),
                            sgT0[p][:, :, :, 0, :]
                            .rearrange("q m h a -> q (m h) a")[:, :, None, :]
                            .to_broadcast([128, 4, A, A]),
                        )
                    nc.vector.tensor_add(rr[:], kkt[:, :, 0], kkt[:, :, 1])
                    nc.scalar.copy(
                        rrT[:], rr[:].rearrange("q n i j -> q n j i")
                    )
                    nc.vector.tensor_mul(
                        uu[:, 2:4].rearrange("q m h r i j -> q (m h r) i j"),
                        va(sgT0[1][:], 8, 0),
                        va(sgT0[1][:], 8, 1),
                    )
                else:
                    # one mul covers both sr (x) sr and si (x) si (ri-blocks)
                    nc.vector.tensor_mul(
                        uu[:].rearrange("q m h r i j -> q (m h r) i j"),
                        va(sgT1[:], 4 * M, 0),
                        va(sgT1[:], 4 * M, 1),
                    )
                    # one merged cross-product mul si (x) sr over (m h)
                    nc.vector.tensor_mul(
                        kkt[:].rearrange("q m h i j -> q (m h) i j"),
                        sgT1[:, :, :, 1, :]
                        .rearrange("q m h a -> q (m h) a")[:, :, :, None]
                        .to_broadcast([128, 2 * M, A, A]),
                        sgT1[:, :, :, 0, :]
                        .rearrange("q m h a -> q (m h) a")[:, :, None, :]
                        .to_broadcast([128, 2 * M, A, A]),
                    )
                    nc.vector.tensor_add(rr[:], kkt[:, :, 0], kkt[:, :, 1])
                    nc.scalar.copy(
                        rrT[:], rr[:].rearrange("q n i j -> q n j i")
                    )
                # dense ri/h-sums hit the fp16 packed modes
                nc.vector.tensor_add(
                    w0[:],
                    uu[:, :, :, 0].rearrange("q m h i j -> q m h (i j)"),
                    uu[:, :, :, 1].rearrange("q m h i j -> q m h (i j)"),
                )
                # Per-half final writes + per-plane DMAs so stores overlap the
                # next half's ops; out[b, ri, 8q + 2m + e, :] = f*[b][q, m]
                # (e-fanout via a stride-0 source dim in the DMA).
                # im rides the identity fim = R - R^T (transpose distributes
                # over the h-sum).
                H2 = M // 2
                dstr = out[b, 0].rearrange(
                    "(q n e) c -> q n (e c)", q=128, n=M, e=2
                )
                dsti = out[b, 1].rearrange(
                    "(q n e) c -> q n (e c)", q=128, n=M, e=2
                )
                qa = (nc.sync, nc.scalar)
                for g in range(2):
                    lo, hi = g * H2, (g + 1) * H2
                    # re-plane DMAs issue (and stream) while the im-plane
                    # subtract still runs, so only the im stores trail the
                    # final DVE op.
                    nc.vector.tensor_add(
                        fre_t[b][:, lo:hi], vh(w0, 0, lo, hi), vh(w0, 1, lo, hi)
                    )
                    for n in range(lo, hi):
                        qa[(b + n) % 2].dma_start(
                            out=dstr[:, n],
                            in_=fre_t[b][:, n, None, :].to_broadcast(
                                [128, 2, A * A]
                            ),
                        )
                    nc.vector.tensor_sub(
                        fim_t[b][:, lo:hi],
                        rr[:, lo:hi].rearrange("q n i j -> q n (i j)"),
                        rrT[:, lo:hi].rearrange("q n i j -> q n (i j)"),
                    )
                    for n in range(lo, hi):
                        qa[(b + n + 1) % 2].dma_start(
                            out=dsti[:, n],
                            in_=fim_t[b][:, n, None, :].to_broadcast(
                                [128, 2, A * A]
                            ),
                        )
    nc.finalize()
    return nc


def _build_generic(est, closest):
    """Generic program: host-gathered sig^T comes in as an input; the whole
    segment-mean + NN-gather is one dense weight matmul on the PE.
      cov[s, (i,j)] = sum_p wt[p, s] * G[p, (i,j)],  G from sig outer products.
    """
    import concourse.bacc as bacc
    import concourse.mybir as mybir
    from concourse.tile import TileContext

    f32 = mybir.dt.float32
    f16 = mybir.dt.float16
    nc = bacc.Bacc(trn_type="TRN2", target_bir_lowering=False)
    # sig^T per batch: [P_EST, A] split as [KP=8, 128, A]
    sgr = nc.declare_dram_parameter("sgr", [B_LOC, P_EST // 128, 128, A], f32, isOutput=False)
    sgi = nc.declare_dram_parameter("sgi", [B_LOC, P_EST // 128, 128, A], f32, isOutput=False)
    wt = nc.declare_dram_parameter("wt", [P_EST, S], f32, isOutput=False)
    out = nc.declare_dram_parameter("out", [B_LOC, 2, S, A * A], f16, isOutput=True)

    KP = P_EST // 128  # contraction chunks
    MS = S // 128      # output subcarrier chunks

    with TileContext(nc) as tc:
        with (
            tc.tile_pool(name="w", bufs=1) as wp,
            tc.tile_pool(name="sig", bufs=2) as sigp,
            tc.tile_pool(name="g", bufs=4) as gp,
            tc.tile_pool(name="ps", bufs=8, space="PSUM") as psp,
            tc.tile_pool(name="f", bufs=2) as fp,
        ):
            w_all = wp.tile([128, KP, S], f32, name="w_all")
            nc.sync.dma_start(
                out=w_all[:], in_=wt[:].rearrange("(k q) s -> q k s", k=KP, q=128)
            )
            for b in range(B_LOC):
                sr = sigp.tile([128, KP, A], f32, tag="sr")
                si = sigp.tile([128, KP, A], f32, tag="si")
                nc.sync.dma_start(
                    out=sr[:], in_=sgr[b].rearrange("k q a -> q k a")
                )
                nc.sync.dma_start(
                    out=si[:], in_=sgi[b].rearrange("k q a -> q k a")
                )

                f = fp.tile([128, 2, MS, A * A], f16, tag="f")
                gtiles = {}
                for k in range(KP):
                    def ii(x):
                        return x[:, k, :, None].to_broadcast([128, A, A])

                    def jj(x):
                        return x[:, k, None, :].to_broadcast([128, A, A])

                    gr = gp.tile([128, A, A], f32, tag=f"gr{k}")
                    gi = gp.tile([128, A, A], f32, tag=f"gi{k}")
                    tt = gp.tile([128, A, A], f32, tag="tt")
                    nc.vector.tensor_mul(gr[:], ii(sr), jj(sr))
                    nc.vector.tensor_mul(tt[:], ii(si), jj(si))
                    nc.vector.tensor_add(gr[:], gr[:], tt[:])
                    nc.vector.tensor_mul(gi[:], ii(si), jj(sr))
                    nc.vector.tensor_mul(tt[:], ii(sr), jj(si))
                    nc.vector.tensor_sub(gi[:], gi[:], tt[:])
                    gtiles[k] = (gr, gi)

                for m in range(MS):
                    for part in range(2):
                        pp = psp.tile([128, A * A], f32, tag="pp")
                        for k in range(KP):
                            g = gtiles[k][part]
                            nc.tensor.matmul(
                                pp[:],
                                lhsT=w_all[:, k, m * 128 : (m + 1) * 128],
                                rhs=g[:].rearrange("q i j -> q (i j)"),
                                start=(k == 0),
                                stop=(k == KP - 1),
                            )
                        nc.vector.tensor_copy(f[:, part, m, :], pp[:])

                dst = out[b].rearrange(
                    "ri (m q) ij -> q ri m ij", m=MS, q=128
                )
                nc.sync.dma_start(out=dst, in_=f[:])
    nc.finalize()
    return nc


def _get_program(est, closest):
    key = (est.tobytes(), closest.tobytes())
    hit = _cache.get(key)
    if hit is not None:
        return hit
    fast = _fast_path_info(est, closest)
    if fast is not None:
        prog = ("fast", _build_fast(*fast), None)
    else:
        counts = np.zeros(S, dtype=np.float64)
        np.add.at(counts, est[:, 1], 1.0)
        denom = np.maximum(counts, 1.0)
        # wt[p, s] = [sc_p == closest[s]] / denom[closest[s]]
        wt = (
            (est[:, 1][:, None] == closest[None, :]).astype(np.float32)
            / denom[closest][None, :].astype(np.float32)
        )
        prog = ("generic", _build_generic(est, closest), np.ascontiguousarray(wt))
    _cache[key] = prog
    return prog


def _make_in_maps(inputs, est, kind, wt):
    yr = np.ascontiguousarray(np.asarray(inputs["y_real"], dtype=np.float32)[:, 0])
    yi = np.ascontiguousarray(np.asarray(inputs["y_imag"], dtype=np.float32)[:, 0])
    if kind == "fast":
        sym0, sym1 = int(est[0, 0]), int(est[P_EST // 2, 0])
        # pack [B, m, (ri h a), q] fp16: the two pilot-symbol slabs at the
        # even (estimated) subcarriers, sqrt(1/2)-prescaled, reordered into
        # per-m DMA-transpose blocks (s' = 4q + m)
        sl = np.stack(
            [yr[:, :, (sym0, sym1), 0::2], yi[:, :, (sym0, sym1), 0::2]],
            axis=1,
        ) * np.float32(0.7071067811865476)  # [B, ri, A, h, 512]
        y2t = np.ascontiguousarray(
            sl.reshape(B, 2, A, 2, 128, 4)
            .transpose(0, 5, 3, 1, 2, 4)  # [B, m, h, ri, A, q]
            .reshape(B, 4, 64, 128)
            .astype(np.float16)
        )
        return [
            {"y2t": y2t[c * B_LOC : (c + 1) * B_LOC]} for c in range(N_CORES)
        ]
    sym = est[:, 0].astype(np.int64)
    sc = est[:, 1].astype(np.int64)
    # host gather: sig[b, a, p] = y[b, a, sym_p, sc_p]
    sgr = yr[:, :, sym, sc]  # [B, A, P]
    sgi = yi[:, :, sym, sc]
    # -> [B, KP, 128, A]
    sgr = np.ascontiguousarray(
        sgr.transpose(0, 2, 1).reshape(B, P_EST // 128, 128, A)
    )
    sgi = np.ascontiguousarray(
        sgi.transpose(0, 2, 1).reshape(B, P_EST // 128, 128, A)
    )
    return [
        {
            "sgr": sgr[c * B_LOC : (c + 1) * B_LOC],
            "sgi": sgi[c * B_LOC : (c + 1) * B_LOC],
            "wt": wt,
        }
        for c in range(N_CORES)
    ]


def kernel(y_real, y_imag, estimation_indices, closest_subcarrier):
    from concourse.bass_utils import run_bass_kernel_spmd

    assert y_real.shape == (B, R, A, T, S), y_real.shape
    est = np.asarray(estimation_indices)
    closest = np.asarray(closest_subcarrier)
    kind, nc, wt = _get_program(est, closest)
    in_maps = _make_in_maps(
        {"y_real": y_real, "y_imag": y_imag}, est, kind, wt
    )

    res = run_bass_kernel_spmd(nc, in_maps, list(range(N_CORES)))
    parts = [np.asarray(res.results[c]["out"]) for c in range(N_CORES)]
    full = np.concatenate(parts, axis=0)  # [B, 2, S, A*A] fp16 (re, im planes)
    cov = np.empty((B, S, A * A), dtype=np.complex64)
    cov.real = full[:, 0]
    cov.imag = full[:, 1]
    cov = cov.reshape(B, R, 1, S, A, A)
    # The per-symbol covariance is t-independent: broadcast over T as a view.
    return np.broadcast_to(cov, (B, R, T, S, A, A))


# revision 68
# speedup vs baseline: 1.0532x; 1.0228x over previous
"""Trainium2 Bass kernel for nn_CovarianceEstimator.

Computes, for y [B=16, R=1, A=16, T=14, S=1024] complex (given as separate
real/imag f32 tensors):
  - gather P=1024 pilot positions (sym_p, sc_p) from estimation_indices
  - per-position A x A outer products sig_p sig_p^H
  - unsorted-segment-mean over subcarrier ids sc_p
  - nearest-neighbor expand via closest_subcarrier to all S subcarriers
  - broadcast over T symbols
Output: [B, R, T, S, A, A] complex64.

Sharding: data-parallel over batch; 2 batches per core on 8 cores.

The reference's trailing broadcast_to over OFDM symbols is a zero-FLOP
replication (every t gets the same [S, A, A] covariance), so the device
computes and writes the covariance once per (batch, subcarrier) --
[B_LOC, 2(re/im), S, A*A] fp16 planes -- and the host assembles complex64
and returns a stride-0 numpy broadcast view over T.  This mirrors the
input side, where the host packs only the two pilot-symbol slabs instead
of shipping all 14 symbols to the device.

Device pipeline per batch b:
  sgT[b][q, m, (h ri a)] = sqrt(1/2) y[b, ., sym_h, 8q+2m]  (DMA-transpose,
      host-packed m-major [64, 128] fp16 blocks through the HWDGE xbar --
      no PE transpose, PSUM or ACT pass; batch 0 lands in per-m-pair tiles
      with queue-alternated blocks so its first mul starts after only two)
  fre/fim[b][q, m, (i j)] = cov(s' = 4q + m)                (DVE)
  out[b, ri, 8q + 2m + e, :] = f*[b][q, m]                  (DMA)
DVE math: outer products as fp16 1x ops (one merged (m h ri)-row mul for
the re-part pair, (m h)-merged muls for the si (x) sr cross products --
the (h, ri, a) payload row order keeps (m h) compact after slicing ri);
planar re/im output keeps every final write dense so the ri/h-sum adds hit
the packed 2x/4x modes; the imaginary part uses fim = R - R^T (R = K_h0 +
K_h1, transposed-AP read); the nearest-neighbor duplication e and the T
broadcast are not materialised on device (e rides a stride-0 DMA source
dim, T a host broadcast view).

Two device-program builders:
  * fast path  - used when the index tensors match the PilotPattern structure
                 (meshgrid of 2 symbols x every-2nd-subcarrier, closest = even
                 floor).
  * generic    - any estimation_indices / closest_subcarrier.  Host folds the
                 whole segment-mean + NN-gather into one dense [S, P] weight
                 matrix applied on the tensor engine.
"""

import numpy as np

B, R, A, T, S = 16, 1, 16, 14, 1024
P_EST = 1024          # number of (sym, sc) estimation positions
N_CORES = 8
B_LOC = B // N_CORES  # 2 batches per core
AA2 = A * A * 2       # interleaved (re, im) row payload per subcarrier

_cache = {}


def _fast_path_info(est, closest):
    """Return (sym0, sym1) if indices match the pilot-pattern structure:
    est == meshgrid([sym0, sym1], arange(0, S, 2)) row-major and
    closest == 2*(arange(S)//2).  Else None."""
    if est.shape != (P_EST, 2) or closest.shape != (S,):
        return None
    sc = np.arange(0, S, 2, dtype=est.dtype)
    if not np.array_equal(est[: S // 2, 1], sc):
        return None
    if not np.array_equal(est[S // 2 :, 1], sc):
        return None
    sym0 = int(est[0, 0])
    sym1 = int(est[S // 2, 0])
    if not (0 <= sym0 < T and 0 <= sym1 < T):
        return None
    if sym1 <= sym0:
        return None  # keep the pilot symbols ordered; generic path covers the rest
    if not np.all(est[: S // 2, 0] == sym0):
        return None
    if not np.all(est[S // 2 :, 0] == sym1):
        return None
    if not np.array_equal(closest, (2 * (np.arange(S) // 2)).astype(closest.dtype)):
        return None
    return sym0, sym1


def _build_fast(sym0, sym1):
    import concourse.bacc as bacc
    import concourse.mybir as mybir
    from concourse.tile import TileContext

    f32 = mybir.dt.float32
    f16 = mybir.dt.float16
    nc = bacc.Bacc(trn_type="TRN2", target_bir_lowering=False)
    # y2t: host-packed, sqrt(1/2)-prescaled pilot slabs in transpose-ready
    # m-major blocks: [b, m, (h ri a), q] with subcarrier s' = 4q + m.
    # Row order (h, ri, a): slicing ri keeps (m h) memory-compact, so the
    # cross-product muls stay merged over (m h).
    y2t = nc.declare_dram_parameter("y2t", [B_LOC, 4, 64, 128], f16, isOutput=False)
    # out is (re, im)-planar: [b, ri, s, (i j)].  Planar keeps every DVE
    # write dense step-1 (the interleaved (ij ri) layout forces 1x mode);
    # the host assembles complex64 from the two planes.
    out = nc.declare_dram_parameter("out", [B_LOC, 2, S, A * A], f16, isOutput=True)

    M = S // 256    # 4 even-subcarrier pairs per partition

    with TileContext(nc) as tc:
        with (
            tc.tile_pool(name="const", bufs=1) as cp,
            tc.tile_pool(name="u", bufs=2) as up,
            tc.tile_pool(name="f", bufs=1) as fp,
        ):
            # DMA-transpose (HWDGE xbar) each m-block [64, 128] straight from
            # DRAM into the sig tile [q, m, ri, h, a]: subcarrier s' = 4q + m
            # lands in partition q with the (ri h a) row payload as columns.
            # No PE transpose, no PSUM, no ACT scale pass (sqrt(1/2) is
            # host-applied); batch-0 blocks split across both HWDGE queues.
            # Batch 0 lands in per-m-pair tiles with queue-alternated blocks,
            # so its first product mul starts after only two transposes; the
            # Tile dependency is then exact (tile granularity).
            sgT0 = [
                cp.tile([128, 2, 2, 2, A], f16, name=f"sgT0_{p}")
                for p in range(2)
            ]
            sgT1 = cp.tile([128, M, 2, 2, A], f16, name="sgT1")
            for m in range(M):
                (nc.sync, nc.scalar)[m % 2].dma_start(
                    out=sgT0[m // 2][:, m % 2],
                    in_=y2t[0, m],
                    transpose=True,
                )
            for m in range(M):
                (nc.sync, nc.scalar)[m % 2].dma_start(
                    out=sgT1[:, m],
                    in_=y2t[1, m],
                    transpose=True,
                )

            # DVE outer products (fp16).  Per batch, all 4 m's in one set of
            # merged ops [q, (h m), A, A]:
            #   re: u0 = sr (x) sr, u1 = si (x) si; w = u0 + u1 (2x);
            #       fre = w[h0] + w[h1]   (dense packed write)
            #   im: K = si (x) sr; R = K[h0] + K[h1] (2x);
            #       fim = R - R^T        (transposed-AP read, 1x)
            # The nearest-neighbor duplication is NOT materialised in SBUF:
            # the output DMA fans each row out to both subcarriers with a
            # stride-0 source dim, which keeps the final DVE writes dense
            # (packed mode) and halves their size.
            fre_t = [
                fp.tile([128, M, A * A], f16, name=f"fre{b}")
                for b in range(B_LOC)
            ]
            fim_t = [
                fp.tile([128, M, A * A], f16, name=f"fim{b}")
                for b in range(B_LOC)
            ]
            def va(x, rows, axis):  # sig tile view, (m ri h) flattened
                sh = [128, rows, A, A]
                v = x.rearrange("q m r h a -> q (m r h) a")
                return (
                    v[:, :, :, None].to_broadcast(sh)
                    if axis == 0
                    else v[:, :, None, :].to_broadcast(sh)
                )

            def vh(x, h, lo, hi):  # h-slice of w0 [q, m, h, (i j)]
                return x[:, lo:hi, h]

            for b in range(B_LOC):
                uu = up.tile([128, M, 2, 2, A, A], f16, tag="uu")
                kkt = up.tile([128, M, 2, A, A], f16, tag="kkt")
                w0 = up.tile([128, M, 2, A * A], f16, tag="w0")
                rr = up.tile([128, M, A, A], f16, tag="rr")
                rrT = up.tile([128, M, A, A], f16, tag="rrT")
                # Emission order computes rr as early as possible so the
                # otherwise-idle ACT engine materialises R^T (strided
                # transposed copy) in parallel with the remaining DVE muls;
                # the fim subtracts then run dense in packed mode instead of
                # paying the 1.43x transposed-read penalty on DVE.
                if b == 0:
                    # per-pair muls: the first starts after only two
                    # transpose blocks have landed
                    nc.vector.tensor_mul(
                        uu[:, 0:2].rearrange("q m h r i j -> q (m h r) i j"),
                        va(sgT0[0][:], 8, 0),
                        va(sgT0[0][:], 8, 1),
                    )
                    for p in range(2):
                        nc.vector.tensor_mul(
                            kkt[:, 2 * p : 2 * p + 2].rearrange(
                                "q m h i j -> q (m h) i j"
                            ),
                            sgT0[p][:, :, :, 1, :]
                            .rearrange("q m h a -> q (m h) a")[:, :, :, None]
                            .to_broadcast([128, 4, A, A]),
                            sgT0[p][:, :, :, 0, :]
                            .rearrange("q m h a -> q (m h) a")[:, :, None, :]
                            .to_broadcast([128, 4, A, A]),
                        )
                    nc.vector.tensor_add(rr[:], kkt[:, :, 0], kkt[:, :, 1])
                    nc.scalar.copy(
                        rrT[:], rr[:].rearrange("q n i j -> q n j i")
                    )
                    nc.vector.tensor_mul(
                        uu[:, 2:4].rearrange("q m h r i j -> q (m h r) i j"),
                        va(sgT0[1][:], 8, 0),
                        va(sgT0[1][:], 8, 1),
                    )
                else:
                    # kk/rr first so the ACT R^T copy hides under the big
                    # U-mul that follows
                    nc.vector.tensor_mul(
                        kkt[:].rearrange("q m h i j -> q (m h) i j"),
                        sgT1[:, :, :, 1, :]
                        .rearrange("q m h a -> q (m h) a")[:, :, :, None]
                        .to_broadcast([128, 2 * M, A, A]),
                        sgT1[:, :, :, 0, :]
                        .rearrange("q m h a -> q (m h) a")[:, :, None, :]
                        .to_broadcast([128, 2 * M, A, A]),
                    )
                    nc.vector.tensor_add(rr[:], kkt[:, :, 0], kkt[:, :, 1])
                    nc.scalar.copy(
                        rrT[:], rr[:].rearrange("q n i j -> q n j i")
                    )
                    # one mul covers both sr (x) sr and si (x) si (ri-blocks)
                    nc.vector.tensor_mul(
                        uu[:].rearrange("q m h r i j -> q (m h r) i j"),
                        va(sgT1[:], 4 * M, 0),
                        va(sgT1[:], 4 * M, 1),
                    )
                # dense ri/h-sums hit the fp16 packed modes
                nc.vector.tensor_add(
                    w0[:],
                    uu[:, :, :, 0].rearrange("q m h i j -> q m h (i j)"),
                    uu[:, :, :, 1].rearrange("q m h i j -> q m h (i j)"),
                )
                # Per-half final writes + per-plane DMAs so stores overlap the
                # next half's ops; out[b, ri, 8q + 2m + e, :] = f*[b][q, m]
                # (e-fanout via a stride-0 source dim in the DMA).
                # im rides the identity fim = R - R^T (transpose distributes
                # over the h-sum).
                H2 = M // 2
                dstr = out[b, 0].rearrange(
                    "(q n e) c -> q n (e c)", q=128, n=M, e=2
                )
                dsti = out[b, 1].rearrange(
                    "(q n e) c -> q n (e c)", q=128, n=M, e=2
                )
                qa = (nc.sync, nc.scalar)
                for g in range(2):
                    lo, hi = g * H2, (g + 1) * H2
                    # re-plane DMAs issue (and stream) while the im-plane
                    # subtract still runs, so only the im stores trail the
                    # final DVE op.
                    nc.vector.tensor_add(
                        fre_t[b][:, lo:hi], vh(w0, 0, lo, hi), vh(w0, 1, lo, hi)
                    )
                    for n in range(lo, hi):
                        qa[(b + n) % 2].dma_start(
                            out=dstr[:, n],
                            in_=fre_t[b][:, n, None, :].to_broadcast(
                                [128, 2, A * A]
                            ),
                        )
                    nc.vector.tensor_sub(
                        fim_t[b][:, lo:hi],
                        rr[:, lo:hi].rearrange("q n i j -> q n (i j)"),
                        rrT[:, lo:hi].rearrange("q n i j -> q n (i j)"),
                    )
                    for n in range(lo, hi):
                        qa[(b + n + 1) % 2].dma_start(
                            out=dsti[:, n],
                            in_=fim_t[b][:, n, None, :].to_broadcast(
                                [128, 2, A * A]
                            ),
                        )
    nc.finalize()
    return nc


def _build_generic(est, closest):
    """Generic program: host-gathered sig^T comes in as an input; the whole
    segment-mean + NN-gather is one dense weight matmul on the PE.
      cov[s, (i,j)] = sum_p wt[p, s] * G[p, (i,j)],  G from sig outer products.
    """
    import concourse.bacc as bacc
    import concourse.mybir as mybir
    from concourse.tile import TileContext

    f32 = mybir.dt.float32
    f16 = mybir.dt.float16
    nc = bacc.Bacc(trn_type="TRN2", target_bir_lowering=False)
    # sig^T per batch: [P_EST, A] split as [KP=8, 128, A]
    sgr = nc.declare_dram_parameter("sgr", [B_LOC, P_EST // 128, 128, A], f32, isOutput=False)
    sgi = nc.declare_dram_parameter("sgi", [B_LOC, P_EST // 128, 128, A], f32, isOutput=False)
    wt = nc.declare_dram_parameter("wt", [P_EST, S], f32, isOutput=False)
    out = nc.declare_dram_parameter("out", [B_LOC, 2, S, A * A], f16, isOutput=True)

    KP = P_EST // 128  # contraction chunks
    MS = S // 128      # output subcarrier chunks

    with TileContext(nc) as tc:
        with (
            tc.tile_pool(name="w", bufs=1) as wp,
            tc.tile_pool(name="sig", bufs=2) as sigp,
            tc.tile_pool(name="g", bufs=4) as gp,
            tc.tile_pool(name="ps", bufs=8, space="PSUM") as psp,
            tc.tile_pool(name="f", bufs=2) as fp,
        ):
            w_all = wp.tile([128, KP, S], f32, name="w_all")
            nc.sync.dma_start(
                out=w_all[:], in_=wt[:].rearrange("(k q) s -> q k s", k=KP, q=128)
            )
            for b in range(B_LOC):
                sr = sigp.tile([128, KP, A], f32, tag="sr")
                si = sigp.tile([128, KP, A], f32, tag="si")
                nc.sync.dma_start(
                    out=sr[:], in_=sgr[b].rearrange("k q a -> q k a")
                )
                nc.sync.dma_start(
                    out=si[:], in_=sgi[b].rearrange("k q a -> q k a")
                )

                f = fp.tile([128, 2, MS, A * A], f16, tag="f")
                gtiles = {}
                for k in range(KP):
                    def ii(x):
                        return x[:, k, :, None].to_broadcast([128, A, A])

                    def jj(x):
                        return x[:, k, None, :].to_broadcast([128, A, A])

                    gr = gp.tile([128, A, A], f32, tag=f"gr{k}")
                    gi = gp.tile([128, A, A], f32, tag=f"gi{k}")
                    tt = gp.tile([128, A, A], f32, tag="tt")
                    nc.vector.tensor_mul(gr[:], ii(sr), jj(sr))
                    nc.vector.tensor_mul(tt[:], ii(si), jj(si))
                    nc.vector.tensor_add(gr[:], gr[:], tt[:])
                    nc.vector.tensor_mul(gi[:], ii(si), jj(sr))
                    nc.vector.tensor_mul(tt[:], ii(sr), jj(si))
                    nc.vector.tensor_sub(gi[:], gi[:], tt[:])
                    gtiles[k] = (gr, gi)

                for m in range(MS):
                    for part in range(2):
                        pp = psp.tile([128, A * A], f32, tag="pp")
                        for k in range(KP):
                            g = gtiles[k][part]
                            nc.tensor.matmul(
                                pp[:],
                                lhsT=w_all[:, k, m * 128 : (m + 1) * 128],
                                rhs=g[:].rearrange("q i j -> q (i j)"),
                                start=(k == 0),
                                stop=(k == KP - 1),
                            )
                        nc.vector.tensor_copy(f[:, part, m, :], pp[:])

                dst = out[b].rearrange(
                    "ri (m q) ij -> q ri m ij", m=MS, q=128
                )
                nc.sync.dma_start(out=dst, in_=f[:])
    nc.finalize()
    return nc


def _get_program(est, closest):
    key = (est.tobytes(), closest.tobytes())
    hit = _cache.get(key)
    if hit is not None:
        return hit
    fast = _fast_path_info(est, closest)
    if fast is not None:
        prog = ("fast", _build_fast(*fast), None)
    else:
        counts = np.zeros(S, dtype=np.float64)
        np.add.at(counts, est[:, 1], 1.0)
        denom = np.maximum(counts, 1.0)
        # wt[p, s] = [sc_p == closest[s]] / denom[closest[s]]
        wt = (
            (est[:, 1][:, None] == closest[None, :]).astype(np.float32)
            / denom[closest][None, :].astype(np.float32)
        )
        prog = ("generic", _build_generic(est, closest), np.ascontiguousarray(wt))
    _cache[key] = prog
    return prog


def _make_in_maps(inputs, est, kind, wt):
    yr = np.ascontiguousarray(np.asarray(inputs["y_real"], dtype=np.float32)[:, 0])
    yi = np.ascontiguousarray(np.asarray(inputs["y_imag"], dtype=np.float32)[:, 0])
    if kind == "fast":
        sym0, sym1 = int(est[0, 0]), int(est[P_EST // 2, 0])
        # pack [B, m, (ri h a), q] fp16: the two pilot-symbol slabs at the
        # even (estimated) subcarriers, sqrt(1/2)-prescaled, reordered into
        # per-m DMA-transpose blocks (s' = 4q + m)
        sl = np.stack(
            [yr[:, :, (sym0, sym1), 0::2], yi[:, :, (sym0, sym1), 0::2]],
            axis=1,
        ) * np.float32(0.7071067811865476)  # [B, ri, A, h, 512]
        y2t = np.ascontiguousarray(
            sl.reshape(B, 2, A, 2, 128, 4)
            .transpose(0, 5, 3, 1, 2, 4)  # [B, m, h, ri, A, q]
            .reshape(B, 4, 64, 128)
            .astype(np.float16)
        )
        return [
            {"y2t": y2t[c * B_LOC : (c + 1) * B_LOC]} for c in range(N_CORES)
        ]
    sym = est[:, 0].astype(np.int64)
    sc = est[:, 1].astype(np.int64)
    # host gather: sig[b, a, p] = y[b, a, sym_p, sc_p]
    sgr = yr[:, :, sym, sc]  # [B, A, P]
    sgi = yi[:, :, sym, sc]
    # -> [B, KP, 128, A]
    sgr = np.ascontiguousarray(
        sgr.transpose(0, 2, 1).reshape(B, P_EST // 128, 128, A)
    )
    sgi = np.ascontiguousarray(
        sgi.transpose(0, 2, 1).reshape(B, P_EST // 128, 128, A)
    )
    return [
        {
            "sgr": sgr[c * B_LOC : (c + 1) * B_LOC],
            "sgi": sgi[c * B_LOC : (c + 1) * B_LOC],
            "wt": wt,
        }
        for c in range(N_CORES)
    ]


def kernel(y_real, y_imag, estimation_indices, closest_subcarrier):
    from concourse.bass_utils import run_bass_kernel_spmd

    assert y_real.shape == (B, R, A, T, S), y_real.shape
    est = np.asarray(estimation_indices)
    closest = np.asarray(closest_subcarrier)
    kind, nc, wt = _get_program(est, closest)
    in_maps = _make_in_maps(
        {"y_real": y_real, "y_imag": y_imag}, est, kind, wt
    )

    res = run_bass_kernel_spmd(nc, in_maps, list(range(N_CORES)))
    parts = [np.asarray(res.results[c]["out"]) for c in range(N_CORES)]
    full = np.concatenate(parts, axis=0)  # [B, 2, S, A*A] fp16 (re, im planes)
    cov = np.empty((B, S, A * A), dtype=np.complex64)
    cov.real = full[:, 0]
    cov.imag = full[:, 1]
    cov = cov.reshape(B, R, 1, S, A, A)
    # The per-symbol covariance is t-independent: broadcast over T as a view.
    return np.broadcast_to(cov, (B, R, T, S, A, A))
